# revision 57
# baseline (speedup 1.0000x reference)
"""Trainium2 Bass kernel for nn_LunaCausalAttention.

Sharding: 8 cores; core c handles batch b = c//4 and heads hs = 4*(c%4) .. hs+4.
Each core: feature-major bf16 projections, then a chunked two-pass causal
linear attention processing all 4 heads per 128-token chunk with fused
(128,256) activations, block-diagonal pq/S/T so state products are single
full-K matmuls, PSUM-resident S/T states, and a partial output projection.
Projection / out-proj matmul groups are interleaved between attention stages
as PE filler so the tensor engine never idles (keeps the p-state ramped).
Host sums the 4 partials per batch and adds bo.
"""
import numpy as np

import concourse.bass as bass
import concourse.mybir as mybir
import concourse.tile as tile
from concourse import bacc
from concourse.masks import make_upper_triangular, make_identity
from concourse.bass_utils import run_bass_kernel_spmd

# static shapes
B, N, D, M, H, DH = 2, 1024, 1024, 64, 16, 64
C = 128                 # token chunk
NCH = N // C            # 8 chunks
NCORES = 8
HPC = 4                 # heads per core
E = HPC * DH            # 256 per-core head features
NF = D // 128           # 8 contraction tiles
BETA = float(np.log(2.0))
SCALE = DH ** -0.5

F32 = mybir.dt.float32
BF16 = mybir.dt.bfloat16
FP8 = mybir.dt.float8e4
DR = mybir.MatmulPerfMode.DoubleRow
SQ = float(2 ** 12)     # wq pre-scale (fp8 subnormal avoidance)
SO = float(2 ** 9)      # wk/wpc/wv pre-scale
import os as _os
KF8 = _os.environ.get("KF8", "qkc")   # which projections run fp8
ADT = BF16              # attention-core operand dtype
AF = mybir.ActivationFunctionType
MUL = mybir.AluOpType.mult
XAX = mybir.AxisListType.X


def build_bass(phase=3):
    nc = bacc.Bacc(None, target_bir_lowering=False)

    # ---- I/O ----
    xT_d = nc.dram_tensor("xT", [D, N], FP8, kind="ExternalInput")       # query[b].T
    xTb_d = nc.dram_tensor("xTb", [D, N], BF16, kind="ExternalInput")    # bf16 copy
    pT_d = nc.dram_tensor("pT", [D, M], BF16, kind="ExternalInput")       # p[b].T
    wq_d = nc.dram_tensor("wq", [D, E], FP8 if "q" in KF8 else BF16, kind="ExternalInput")       # scale folded
    wk_d = nc.dram_tensor("wk", [D, E], FP8 if "k" in KF8 else BF16, kind="ExternalInput")
    wv_d = nc.dram_tensor("wv", [D, E], FP8 if "v" in KF8 else BF16, kind="ExternalInput")
    wpc_d = nc.dram_tensor("wpc", [D, E], FP8 if "c" in KF8 else BF16, kind="ExternalInput")
    wpq_d = nc.dram_tensor("wpq", [D, E], BF16, kind="ExternalInput")     # scale folded
    wo_d = nc.dram_tensor("wo", [E, D], BF16, kind="ExternalInput")
    bq_d = nc.dram_tensor("bq", [128, 2], F32, kind="ExternalInput")     # [i,et]=b[128et+i]
    bk_d = nc.dram_tensor("bk", [128, 2], F32, kind="ExternalInput")
    bpc_d = nc.dram_tensor("bpc", [128, 2], F32, kind="ExternalInput")
    bpq_d = nc.dram_tensor("bpq", [128, 2], F32, kind="ExternalInput")
    bvr_d = nc.dram_tensor("bvr", [1, E], BF16, kind="ExternalInput")     # row form
    rc_d = nc.dram_tensor("rc", [128, NCH], F32, kind="ExternalInput")   # 1/((i+1)*beta)
    ones_d = nc.dram_tensor("onesr", [1, 128], BF16, kind="ExternalInput")
    out_d = nc.dram_tensor("outp", [N, D], F32, kind="ExternalOutput")

    with tile.TileContext(nc) as tc:
        with (
            tc.tile_pool(name="singles", bufs=1) as singles,
            tc.tile_pool(name="work", bufs=3) as work,
            tc.tile_pool(name="obuf", bufs=3) as obuf,
            tc.tile_pool(name="psum", bufs=1, space="PSUM") as psum,
        ):
            # ---- constants ----
            triu = singles.tile([128, 4, C], F32)     # four upper-tri copies
            for tcp in range(4):
                make_upper_triangular(nc, triu[:, tcp, :], val=1.0, diag=True)
            identb = singles.tile([128, 128], ADT)
            make_identity(nc, identb)
            ones = singles.tile([1, 128], BF16)
            nc.sync.dma_start(out=ones, in_=ones_d[:, :])

            def load_w(name, dram, dt=BF16):
                w = singles.tile([128, NF, E], dt, name=name)
                nc.sync.dma_start(
                    out=w, in_=dram.rearrange("(f p) e -> p f e", p=128))
                return w

            # DMA in earliest-need order.
            wpq_sb = load_w("wpq_sb", wpq_d)
            pT_sb = singles.tile([128, NF, M], BF16)
            nc.sync.dma_start(
                out=pT_sb, in_=pT_d.rearrange("(f p) m -> p f m", p=128))
            bpq_sb = singles.tile([128, 2], F32)
            nc.sync.dma_start(out=bpq_sb, in_=bpq_d[:, :])
            bpc_sb = singles.tile([128, 2], F32)
            nc.sync.dma_start(out=bpc_sb, in_=bpc_d[:, :])
            bq_sb = singles.tile([128, 2], F32)
            nc.sync.dma_start(out=bq_sb, in_=bq_d[:, :])
            bk_sb = singles.tile([128, 2], F32)
            nc.sync.dma_start(out=bk_sb, in_=bk_d[:, :])
            rc_sb = singles.tile([128, NCH], F32)
            nc.sync.dma_start(out=rc_sb, in_=rc_d[:, :])
            bvr_sb = singles.tile([1, E], BF16)
            nc.sync.dma_start(out=bvr_sb, in_=bvr_d[:, :])
            wpc_sb = load_w("wpc_sb", wpc_d, FP8 if "c" in KF8 else BF16)
            xt4 = singles.tile([128, NF, N], FP8, name="xt4")
            nc.sync.dma_start(
                out=xt4[:, :, 0:256],
                in_=xT_d.rearrange("(f p) n -> p f n", p=128)[:, :, 0:256])
            xtb = None
            if len(KF8) < 4:
                xtb = singles.tile([128, NF, N], BF16, name="xtb")
                nc.sync.dma_start(
                    out=xtb[:, :, 0:256],
                    in_=xTb_d.rearrange("(f p) n -> p f n",
                                        p=128)[:, :, 0:256])
            wq_sb = load_w("wq_sb", wq_d, FP8 if "q" in KF8 else BF16)
            wk_sb = load_w("wk_sb", wk_d, FP8 if "k" in KF8 else BF16)
            wv_sb = load_w("wv_sb", wv_d, FP8 if "v" in KF8 else BF16)
            nc.sync.dma_start(
                out=xt4[:, :, 256:N],
                in_=xT_d.rearrange("(f p) n -> p f n", p=128)[:, :, 256:N])
            if xtb is not None:
                nc.sync.dma_start(
                    out=xtb[:, :, 256:N],
                    in_=xTb_d.rearrange("(f p) n -> p f n",
                                        p=128)[:, :, 256:N])
            wo_sb = singles.tile([128, 2, D], BF16)
            nc.sync.dma_start(
                out=wo_sb, in_=wo_d.rearrange("(t p) o -> p t o", p=128))

            # projection outputs (feature-major; dim1 = head pair et/hp)
            qT_sb = singles.tile([128, 2, N], ADT)
            kT_sb = singles.tile([128, 2, N], ADT)
            pcT_sb = singles.tile([128, 2, N], ADT)
            pq_sb = singles.tile([128, 2, M], ADT)
            bdpq_sb = singles.tile([128, 2, 128], ADT)   # block-diag pq per hp
            vtok_sb = [singles.tile([128, E], ADT, name=f"vtok{t}")
                       for t in range(NCH)]
            attnT_sb = [singles.tile([128, 2, C], ADT, name=f"attnT{t}")
                        for t in range(NCH)]
            S_sb = singles.tile([128, 2, M], ADT)   # compact S (f2, hp, m)
            T_sb = singles.tile([64, 4, DH], ADT)   # compact T (m, 2hp+h, dh)
            nc.gpsimd.memset(S_sb, 0.0)
            nc.gpsimd.memset(T_sb, 0.0)

            # ---- persistent PSUM banks (one bank per tile: the psum group
            # bookkeeping assumes partition stride == one 2KB bank).
            # pGa/pGb: G parity slots [0:256]; G2 [256:512] overlapping the
            # short-lived at/ptT transpose staging (consumed before G2 writes).
            pGa = psum.tile([128, 512], F32, tag="pGa", name="pGa")
            pGb = psum.tile([128, 512], F32, tag="pGb", name="pGb")

            def g_slot(c, hp, h):  # (128,128) f32 G region, bank a/b
                return (pGa, pGb)[h][:, 128 * hp:128 * hp + 128]

            def g2_r(g):    # (128,128) f32 G2 region for head-group g
                return (pGa, pGb)[g // 2][:, 256 + 128 * (g % 2):
                                           384 + 128 * (g % 2)]

            def at_r(g):    # (64,128) ADT region for z^T of head-group g
                return (pGa, pGb)[g // 2][:, 256 + 64 * (g % 2):
                                          320 + 64 * (g % 2)].bitcast(
                                              ADT)[0:64, :]

            def ptT_r(g):   # (64,128) ADT region for P~^T of head-group g
                return (pGa, pGb)[g // 2][:, 384 + 64 * (g % 2):
                                          448 + 64 * (g % 2)].bitcast(
                                              ADT)[0:64, :]

            # ---- emission helpers ----
            def proj_group(dst, w, b, et, nq, beng, desc):
                # one projection quarter: 256 tokens for one 128-feature group
                pp = psum.tile([128, 512], F32, tag="pp", bufs=2, name="pp")
                if w.dtype == FP8:
                    for f in range(0, NF, 2):
                        nc.tensor.matmul(
                            pp[:, 0:256],
                            w[:, f:f + 2, et * 128:(et + 1) * 128],
                            xt4[:, f:f + 2, nq * 256:(nq + 1) * 256],
                            start=(f == 0), stop=(f == NF - 2), perf_mode=DR)
                else:
                    for f in range(NF):
                        nc.tensor.matmul(
                            pp[:, 0:256], w[:, f, et * 128:(et + 1) * 128],
                            xtb[:, f, nq * 256:(nq + 1) * 256],
                            start=(f == 0), stop=(f == NF - 1))
                dsl = dst[:, et, nq * 256:(nq + 1) * 256]
                if beng is nc.scalar:
                    nc.scalar.activation(dsl, pp[:, 0:256], AF.Identity,
                                         bias=b[:, et:et + 1], scale=desc)
                else:
                    beng.tensor_scalar(dsl, pp[:, 0:256], desc,
                                       b[:, et:et + 1], MUL,
                                       mybir.AluOpType.add)

            def vtok_group(tb):
                pkv = psum.tile([128, 512], F32, tag="pp", bufs=2, name="pp")
                if wv_sb.dtype == FP8:
                    for f in range(0, NF, 2):
                        nc.tensor.matmul(
                            pkv[:, 0:E],
                            xt4[:, f:f + 2, tb * 128:(tb + 1) * 128],
                            wv_sb[:, f:f + 2, :], start=(f == 0), stop=False,
                            perf_mode=DR)
                else:
                    for f in range(NF):
                        nc.tensor.matmul(
                            pkv[:, 0:E], xtb[:, f, tb * 128:(tb + 1) * 128],
                            wv_sb[:, f, :], start=(f == 0), stop=False)
                nc.tensor.matmul(pkv[:, 0:E], ones, bvr_sb,
                                 start=False, stop=True)
                nc.scalar.activation(
                    vtok_sb[tb], pkv[:, 0:E], AF.Copy,
                    scale=(1.0 / SO) if wv_sb.dtype == FP8 else 1.0)

            def out_block(c):
                tok = slice(c * C, (c + 1) * C)
                ob = obuf.tile([128, D], F32, name="ob")
                for oh in range(2):
                    po = psum.tile([128, 512], F32, tag="pp", bufs=2, name="pp")
                    for et in range(2):
                        nc.tensor.matmul(
                            po, attnT_sb[c][:, et, :],
                            wo_sb[:, et, oh * 512:(oh + 1) * 512],
                            start=(et == 0), stop=(et == 1))
                    if oh == 0:
                        nc.vector.tensor_copy(ob[:, 0:512], po)
                    else:
                        nc.scalar.activation(ob[:, 512:1024], po, AF.Copy)
                nc.sync.dma_start(out=out_d[tok, :], in_=ob)

            # ---- attention chunk stages (all 4 heads per chunk) ----
            def chunk_stages(c):
                tok = slice(c * C, (c + 1) * C)
                # per-chunk banks: [pz then aw (shared) | S-inc | T-inc
                # | ktokT | pan]
                pzA = psum.tile([128, 512], F32, tag="pzA", bufs=2, name="pzA")
                pzB = psum.tile([128, 512], F32, tag="pzB", bufs=2, name="pzB")
                pzX = (pzA, pzB)
                ktokT_r = (pzA[:, 320:384].bitcast(ADT),
                           pzB[:, 320:384].bitcast(ADT))
                pan_r = (pzA[:, 384:512], pzB[:, 384:512])
                ez4 = work.tile([128, 256], F32, name="ez4")
                z4 = work.tile([128, 2, 128], ADT, name="z4")
                ktok = work.tile([128, 2, 128], ADT, name="ktok")
                atT = work.tile([64, 4, 128], ADT, name="atT")
                gm4 = work.tile([128, 4, 128], ADT, name="gm4")
                g2m4 = work.tile([128, 4, 128], ADT, name="g2m4")
                ex4 = work.tile([128, 256], ADT, name="ex4")
                rs4 = work.tile([128, 4], F32, name="rs4")
                rcp4 = work.tile([128, 4], F32, name="rcp4")
                pt4 = work.tile([128, 2, 128], ADT, name="pt4")
                ptT4 = work.tile([64, 4, 128], ADT, name="ptT4")

                def stA():  # Z path: pz, K transposes, softplus, z^T
                    for hp in (0, 1):
                        nc.tensor.matmul(pzX[hp][:, 0:128],
                                         pcT_sb[:, hp, tok], bdpq_sb[:, hp, :],
                                         start=True, stop=True)
                        nc.tensor.transpose(ktokT_r[hp], kT_sb[:, hp, tok],
                                            identb)
                    for hp in (0, 1):
                        nc.scalar.activation(ez4[:, 128 * hp:128 * hp + 128],
                                             pzX[hp][:, 0:128], AF.Exp,
                                             scale=BETA)
                        nc.scalar.activation(z4[:, hp, :],
                                             ez4[:, 128 * hp:128 * hp + 128],
                                             AF.Ln, bias=1.0)
                    for hp in (0, 1):
                        nc.scalar.activation(ktok[:, hp, :], ktokT_r[hp],
                                             AF.Copy)
                        for h in (0, 1):
                            nc.tensor.transpose(
                                at_r(2 * hp + h),
                                z4[:, hp, 64 * h:64 * h + 64], identb,
                                tile_position=(0, 0))
                    for hb in (0, 1):
                        nc.vector.tensor_copy(
                            atT[:, 2 * hb:2 * hb + 2, :],
                            (pGa, pGb)[hb][:, 256:384].bitcast(ADT)[0:64, :]
                            .rearrange("p (a b) -> p a b", a=2))
                    for hp in (0, 1):
                        for h in (0, 1):
                            s = slice(64 * h, 64 * h + 64)
                            nc.tensor.matmul(g_slot(c, hp, h),
                                             kT_sb[s, hp, tok],
                                             qT_sb[s, hp, tok], start=True,
                                             stop=True,
                                             tile_position=(64 * h, 0))
                    for h in (0, 1):
                        nc.vector.tensor_mul(
                            gm4.rearrange("p (a b) x -> p b a x", b=2)[:, h],
                            (pGa, pGb)[h][:, 0:256].rearrange(
                                "p (a b) -> p a b", a=2),
                            triu[:, 0:2, :])

                def stB():  # aw: masked-G part + q.S state part
                    for h in (0, 1):      # aw for head h -> bank h
                        for hp in (0, 1):
                            nc.tensor.matmul(
                                pzX[h][:, 64 * hp:64 * hp + 64],
                                gm4[:, 2 * hp + h, :],
                                z4[:, hp, 64 * h:64 * h + 64],
                                start=(hp == 0), stop=(c == 0 and hp == 1))
                    if c > 0:
                        for hp in (0, 1):
                            for h in (0, 1):  # adjacent ops alternate banks
                                s = slice(64 * h, 64 * h + 64)
                                nc.tensor.matmul(
                                    pzX[h][:, 64 * hp:64 * hp + 64],
                                    qT_sb[s, hp, tok], S_sb[s, hp, :],
                                    start=False, stop=(hp == 1),
                                    tile_position=(64 * h, 0))

                def stC():  # softmax: exp, rowsums, P~, P~^T
                    for h in (0, 1):   # aw for head h lives in bank h
                        nc.scalar.activation(ex4[:, 128 * h:128 * h + 128],
                                             pzX[h][:, 0:128], AF.Exp,
                                             scale=rc_sb[:, c:c + 1])
                    nc.vector.tensor_reduce(
                        rs4, ex4.rearrange("p (g m) -> p g m", g=4), XAX,
                        mybir.AluOpType.add)
                    nc.vector.reciprocal(rcp4, rs4)
                    for hp in (0, 1):
                        for h in (0, 1):
                            gx = 2 * h + hp
                            nc.vector.tensor_scalar(
                                pt4[:, hp, 64 * h:64 * h + 64],
                                ex4[:, 64 * gx:64 * gx + 64],
                                rcp4[:, gx:gx + 1], rc_sb[:, c:c + 1], MUL,
                                MUL)
                    for hp in (0, 1):
                        for h in (0, 1):
                            nc.tensor.transpose(
                                ptT_r(2 * hp + h),
                                pt4[:, hp, 64 * h:64 * h + 64],
                                identb, tile_position=(0, 0))
                    for hb in (0, 1):
                        nc.vector.tensor_copy(
                            ptT4[:, 2 * hb:2 * hb + 2, :],
                            (pGa, pGb)[hb][:, 384:512].bitcast(ADT)[0:64, :]
                            .rearrange("p (a b) -> p a b", a=2))

                def stD():  # pass 2: G2, attnT (+ T.P~^T state part)
                    for g in range(4):
                        nc.tensor.matmul(g2_r(g), atT[:, g, :], ptT4[:, g, :],
                                         start=True, stop=True,
                                         tile_position=(0, 0))
                    for hb in (0, 1):
                        nc.vector.tensor_mul(
                            g2m4[:, 2 * hb:2 * hb + 2, :],
                            (pGa, pGb)[hb][:, 256:512].rearrange(
                                "p (a b) -> p a b", a=2),
                            triu[:, 0:2, :])
                    for hp in (0, 1):
                        for h in (0, 1):
                            nc.tensor.matmul(
                                pan_r[hp][64 * h:64 * h + 64, :],
                                vtok_sb[c][:, 128 * hp + 64 * h:
                                           128 * hp + 64 * h + 64],
                                g2m4[:, 2 * hp + h, :],
                                start=True, stop=(c == 0),
                                skip_group_check=(h == 1),
                                tile_position=(0, 64 * h))
                    if c > 0:
                        for h in (0, 1):
                            for hp in (0, 1):  # adjacent ops alternate banks
                                g = 2 * hp + h
                                nc.tensor.matmul(
                                    pan_r[hp][64 * h:64 * h + 64, :],
                                    T_sb[:, g, :], ptT4[:, g, :],
                                    start=False, stop=True,
                                    skip_group_check=(h == 1),
                                    tile_position=(0, 64 * h))
                    nc.scalar.activation(attnT_sb[c][:, 0, :],
                                         pan_r[0], AF.Copy)
                    nc.vector.tensor_copy(attnT_sb[c][:, 1, :], pan_r[1])

                def stES():  # S increment + add (before next chunk's QS)
                    if c == NCH - 1:
                        return
                    for hp in (0, 1):
                        psd = pzX[hp][:, 128:192]
                        for h in (0, 1):
                            nc.tensor.matmul(
                                psd[64 * h:64 * h + 64, :],
                                ktok[:, hp, 64 * h:64 * h + 64],
                                z4[:, hp, 64 * h:64 * h + 64],
                                start=True, stop=True,
                                skip_group_check=(h == 1),
                                tile_position=(0, 64 * h))
                    for hp in (0, 1):
                        nc.vector.tensor_add(S_sb[:, hp, :], S_sb[:, hp, :],
                                             pzX[hp][:, 128:192])

                def stET():  # T increment + add (after this chunk's stD)
                    if c == NCH - 1:
                        return
                    for hp in (0, 1):
                        ptd = pzX[hp][:, 192:320]
                        for h in (0, 1):
                            nc.tensor.matmul(
                                ptd[0:64, 64 * h:64 * h + 64],
                                z4[:, hp, 64 * h:64 * h + 64],
                                vtok_sb[c][:, 128 * hp + 64 * h:
                                           128 * hp + 64 * h + 64],
                                start=True, stop=True,
                                tile_position=(0, 0))
                    for hp in (0, 1):
                        nc.vector.tensor_add(
                            T_sb[:, 2 * hp:2 * hp + 2, :],
                            T_sb[:, 2 * hp:2 * hp + 2, :],
                            pzX[hp][0:64, 192:320].rearrange(
                                "p (a b) -> p a b", a=2))

                return [stA, stB, stC, stD, stES, stET]

            # ---- program ----
            # pq projection + block-diag pq
            for et in range(2):
                ppq = psum.tile([128, 512], F32, tag="pp", bufs=2, name="pp")
                for f in range(NF):
                    nc.tensor.matmul(
                        ppq[:, 0:M], wpq_sb[:, f, et * 128:(et + 1) * 128],
                        pT_sb[:, f, :],
                        start=(f == 0), stop=(f == NF - 1))
                nc.vector.tensor_scalar_add(pq_sb[:, et, :], ppq[:, 0:M],
                                            bpq_sb[:, et:et + 1])
            nc.gpsimd.memset(bdpq_sb, 0.0)
            for hp in (0, 1):
                for h in (0, 1):
                    s = slice(64 * h, 64 * h + 64)
                    nc.gpsimd.tensor_copy(bdpq_sb[s, hp, 64 * h:64 * h + 64],
                                          pq_sb[s, hp, :])

            # minimal pre-loop: chunk 0-1 deps; everything else is filler
            PSPECS = (
                (pcT_sb, wpc_sb, bpc_sb, nc.vector,
                 (1.0 / SO) if "c" in KF8 else 1.0),
                (qT_sb, wq_sb, bq_sb, nc.scalar,
                 (1.0 / SQ) if "q" in KF8 else 1.0),
                (kT_sb, wk_sb, bk_sb, nc.vector,
                 (1.0 / SO) if "k" in KF8 else 1.0))
            for (dst, w, b, be, ds) in PSPECS:
                for et in range(2):
                    proj_group(dst, w, b, et, 0, be, ds)
            vtok_group(0)
            vtok_group(1)

            fillers = []   # (deadline_chunk, emit_fn)
            for nq in range(1, 4):
                for (dst, w, b, be, ds) in PSPECS:
                    for et in range(2):
                        fillers.append(
                            (2 * nq,
                             lambda dst=dst, w=w, b=b, et=et, be=be, nq=nq,
                             ds=ds: proj_group(dst, w, b, et, nq, be, ds)))
                for tb in (2 * nq, 2 * nq + 1):
                    fillers.append((tb, lambda tb=tb: vtok_group(tb)))

            if phase >= 2:
                # 2-deep skewed pipeline: A/B of chunk i overlap C/D of i-1,
                # with E(i-1) hoisted before B(i) so the state adds precede
                # the q.S / T.P~ consumers in every engine queue.
                def pop_filler():
                    if fillers:
                        fillers.pop(0)[1]()

                def force_due(i):
                    while fillers and fillers[0][0] <= i:
                        fillers.pop(0)[1]()

                live = {}
                for i in range(NCH + 1):
                    if i < NCH:
                        force_due(i)            # deps of chunk i first
                        live[i] = chunk_stages(i)
                        live[i][0]()            # stA(i)
                        pop_filler()
                        if i > 0:
                            live[i - 1][4]()    # stES(i-1)
                            pop_filler()
                        live[i][1]()            # stB(i)
                        pop_filler()
                    if i > 0:
                        live[i - 1][2]()        # stC(i-1)
                        pop_filler()
                        live[i - 1][3]()        # stD(i-1)
                        live[i - 1][5]()        # stET(i-1)
                        pop_filler()
                        fillers.append((i + 1, lambda c=i - 1: out_block(c)))
                        del live[i - 1]
                while fillers:
                    fillers.pop(0)[1]()

    # Patch the act-table map so Exp and Ln both resolve to the combined
    # natural_log_exp_and_others set (otherwise the load-placement pass
    # alternates exp_and_others <-> natural_log per chunk, ~42us of reloads).
    import concourse.bacc as _bacc_mod
    from concourse.hw_specs import get_activation_tables as _gat
    _orig_gat = _bacc_mod.get_activation_tables

    def _patched_gat(arch):
        t = _gat(arch)
        for name, s in t.items():
            if name != "natural_log_exp_and_others":
                s.discard(AF.Exp)
                s.discard(AF.Ln)
        return t

    _bacc_mod.get_activation_tables = _patched_gat
    try:
        nc.compile()
    finally:
        _bacc_mod.get_activation_tables = _orig_gat
    return nc
